# revision 6
# baseline (speedup 1.0000x reference)
"""DeepseekV3 MLA attention on 8 trn2 NeuronCores.

Sharding: data-parallel over batch (2 groups) x tensor-parallel over heads
(4-way, 8 heads/core). core c handles batch c//4, heads (c%4)*8..+8.
Each core computes the full latent stems for its batch (replicated within
the batch group), its head-slice of q_b/kv_b/attention, and a partial
o_proj output; the host sums the 4 partials per batch.

Everything on-device lives in transposed [dim, token] layout so no PE
transposes are needed anywhere:
  stem:   c_qT = wa^T @ hiddenT        (lhsT=wa chunk, rhs=hiddenT chunk)
  q_b:    qT = qbw^T @ c_qT_scaled
  scores: sT[k,q]: lhsT=kT[d,kblk], rhs=qT[d,qsb]   (PSUM [k,q])
  AV:     outT[vh,q]: lhsT=v[k,vh], rhs=expT[k,q]
  o_proj: o[q,e]: lhsT=attn_outT[c,qblk], rhs=ow[c,e]
Softmax skips the max-subtraction (scores are O(1) here; masked lanes
underflow exp to exactly 0) and the denominator is accumulated with a
ones-column matmul; its reciprocal is broadcast via a DRAM bounce.
All matmuls run in float32r (FP22-truncated fp32), which streams at bf16
rate for moving free dims >= 256.

Head-pair interleaving keeps the SBUF working set under the ~192KB/partition
pool budget; attention outputs bounce through DRAM for the o_proj phase.
"""

import numpy as np

B, S, HID = 2, 1024, 4096
H, NOPE, ROPE, QKH, VH = 32, 128, 64, 192, 128
QLR, KVLR = 1536, 512
EPS = 1e-6
SCALING = QKH ** -0.5
HG = 8            # heads per core
NCORES = 8
QSB = 512         # q superblock (matmul moving free dim)
NQSB = S // QSB   # 2
KB = 128          # k block
NKB = S // KB     # 8
NEG_SKIP = -1e8   # mask values below this => exp == 0 => block skipped

_CACHE = {}


def _mask_plan(mask):
    """Classify each (qsb, kblock) of the additive mask.

    plan[i2][j] is None (no mask), 'skip' (fully masked), or an index into
    patterns (list of [128, QSB] f32 blocks holding maskT/SCALING).
    """
    maskT = np.ascontiguousarray(mask.reshape(S, S).T)  # [k, q]
    patterns = []
    keys = {}
    plan = []
    for i2 in range(NQSB):
        row = []
        for j in range(NKB):
            blk = maskT[j * KB:(j + 1) * KB, i2 * QSB:(i2 + 1) * QSB]
            if np.all(blk == 0.0):
                row.append(None)
            elif np.all(blk <= NEG_SKIP):
                row.append('skip')
            else:
                key = blk.tobytes()
                if key not in keys:
                    keys[key] = len(patterns)
                    patterns.append((blk / SCALING).astype(np.float32))
                row.append(keys[key])
        plan.append(row)
    return plan, patterns


def _build(plan, n_pat):
    import concourse.mybir as mybir
    import concourse.tile as tile
    from concourse import bacc

    F32 = mybir.dt.float32
    F32R = mybir.dt.float32r
    Exp = mybir.ActivationFunctionType.Exp
    Sqrt = mybir.ActivationFunctionType.Sqrt
    Ident = mybir.ActivationFunctionType.Identity
    Square = mybir.ActivationFunctionType.Square
    MUL = mybir.AluOpType.mult
    ADD = mybir.AluOpType.add

    nc = bacc.Bacc("TRN2", target_bir_lowering=False, debug=False,
                   enable_asserts=False, num_devices=NCORES)

    hT_d = nc.dram_tensor("hT", [HID, S], F32R, kind="ExternalInput").ap()
    wa_d = nc.dram_tensor("wa", [HID, QLR + KVLR + ROPE], F32R,
                          kind="ExternalInput").ap()
    # qbw: per pair p: [nope h0 |nope h1 |lo h0 |lo h1 |hi h0 |hi h1] = 384
    qbw_d = nc.dram_tensor("qbw", [QLR, HG * QKH], F32R,
                           kind="ExternalInput").ap()
    kvbn_d = nc.dram_tensor("kvbn", [KVLR, HG * NOPE], F32R,
                            kind="ExternalInput").ap()
    kvbv_d = nc.dram_tensor("kvbv", [KVLR, HG * VH], F32R,
                            kind="ExternalInput").ap()
    ow_d = nc.dram_tensor("ow", [HG * VH, HID], F32R,
                          kind="ExternalInput").ap()
    # trig tables 4x row-tiled so any 32-aligned slice has a matching base
    coslo_d = nc.dram_tensor("coslo", [128, S], F32, kind="ExternalInput").ap()
    coshi_d = nc.dram_tensor("coshi", [128, S], F32, kind="ExternalInput").ap()
    sinlo_d = nc.dram_tensor("sinlo", [128, S], F32, kind="ExternalInput").ap()
    sinhi_d = nc.dram_tensor("sinhi", [128, S], F32, kind="ExternalInput").ap()
    ones_d = nc.dram_tensor("ones", [128, 1], F32R, kind="ExternalInput").ap()
    if n_pat:
        mp_d = nc.dram_tensor("maskpat", [n_pat, 128, QSB], F32,
                              kind="ExternalInput").ap()
    out_d = nc.dram_tensor("outp", [S, HID], F32, kind="ExternalOutput").ap()

    rs_scr = nc.dram_tensor("rs_scr", [2, S], F32, kind="Internal").ap()
    den_scr = nc.dram_tensor("den_scr", [HG * NQSB, QSB], F32,
                             kind="Internal").ap()
    attn_scr = nc.dram_tensor("attn_scr", [HG, 128, S], F32R,
                              kind="Internal").ap()

    NE = HID // 128      # 32 stem contraction chunks
    NRQ = QLR // 128     # 12
    NRKV = KVLR // 128   # 4
    # stem m-tiles: 12 c_qT + 4 c_kvT + 1 krot(64 cols)
    STEM_MG = [list(range(0, 8)), list(range(8, 16)), [16]]

    hT_re = hT_d.rearrange("(e p) q -> p e q", p=128)

    with nc.allow_low_precision(reason="float32r tiles carry fp32 bits"), \
         tile.TileContext(nc) as tc:
      with tc.tile_pool(name="singles", bufs=1) as singles, \
           tc.tile_pool(name="lat", bufs=1) as latp:
        ones_r = singles.tile([128, 1], F32R)
        nc.sync.dma_start(out=ones_r, in_=ones_d)
        eps_t = singles.tile([1, 1], F32)
        nc.vector.memset(eps_t, EPS)
        coslo = singles.tile([128, S], F32)
        coshi = singles.tile([128, S], F32)
        sinlo = singles.tile([128, S], F32)
        sinhi = singles.tile([128, S], F32)
        nc.sync.dma_start(out=coslo, in_=coslo_d)
        nc.sync.dma_start(out=coshi, in_=coshi_d)
        nc.sync.dma_start(out=sinlo, in_=sinlo_d)
        nc.sync.dma_start(out=sinhi, in_=sinhi_d)
        if n_pat:
            maskp = singles.tile([128, n_pat, QSB], F32)
            for p in range(n_pat):
                nc.sync.dma_start(out=maskp[:, p, :], in_=mp_d[p])

        # long-lived latents (raw, fp32r; rms scales folded in later at the
        # projection copybacks since per-token scaling commutes through the
        # r-contraction) + k rope
        cq_raw = [latp.tile([128, S], F32R, name=f"cq_raw{r}")
                  for r in range(NRQ)]
        ckv_raw = [latp.tile([128, S], F32R, name=f"ckv_raw{r}")
                   for r in range(NRKV)]
        krot_raw = latp.tile([64, S], F32)
        krope = latp.tile([64, S], F32R)

        # ---------------- Phase A: stem ----------------
        if True:
            with (
                tc.tile_pool(name="stem_h", bufs=1) as hp,
                tc.tile_pool(name="stem_w", bufs=4) as wap,
                tc.tile_pool(name="stem_ps", bufs=1, space="PSUM") as psst,
            ):
                for i2 in range(NQSB):
                    qs = slice(i2 * QSB, (i2 + 1) * QSB)
                    hT_sb = hp.tile([128, NE, QSB], F32R, tag="ht")
                    for e in range(NE):
                        nc.sync.dma_start(
                            out=hT_sb[:, e, :], in_=hT_re[:, e, qs])
                    for mg in STEM_MG:
                        w = 64 if mg == [16] else 128 * len(mg)
                        col0 = 128 * mg[0]
                        pss = [psst.tile([64 if m == 16 else 128, QSB],
                                         F32, name=f"ps_stem{m}",
                                         tag=f"s{mi}")
                               for mi, m in enumerate(mg)]
                        for e in range(NE):
                            wa_t = wap.tile([128, w], F32R, tag="wa",
                                            name="wa_t")
                            nc.sync.dma_start(
                                out=wa_t,
                                in_=wa_d[e * 128:(e + 1) * 128,
                                         col0:col0 + w])
                            for mi, m in enumerate(mg):
                                lw = 64 if m == 16 else 128
                                nc.tensor.matmul(
                                    pss[mi],
                                    wa_t[:, mi * 128:mi * 128 + lw],
                                    hT_sb[:, e, :],
                                    start=(e == 0), stop=(e == NE - 1))
                        for mi, m in enumerate(mg):
                            if m < NRQ:
                                dst = cq_raw[m][:, qs]
                            elif m < 16:
                                dst = ckv_raw[m - NRQ][:, qs]
                            else:
                                dst = krot_raw[:, qs]
                            nc.vector.tensor_copy(dst, pss[mi])

            # ---------------- Phase B: rmsnorm + fold ----------------
            with (
                tc.tile_pool(name="sq", bufs=3) as sqp,
                tc.tile_pool(name="rms", bufs=4) as rmsp,
                tc.tile_pool(name="rms_ps", bufs=2, space="PSUM") as psss,
            ):
                for side, tiles, dim in ((0, cq_raw, QLR),
                                         (1, ckv_raw, KVLR)):
                    for i2 in range(NQSB):
                        qs = slice(i2 * QSB, (i2 + 1) * QSB)
                        ss = psss.tile([1, QSB], F32, tag="ss", name="ss")
                        for r, t in enumerate(tiles):
                            sq = sqp.tile([128, QSB], F32R, tag="sq",
                                          name="sq")
                            nc.scalar.activation(sq, t[:, qs], Square)
                            nc.tensor.matmul(
                                ss, ones_r, sq,
                                start=(r == 0), stop=(r == len(tiles) - 1))
                        srow = rmsp.tile([1, QSB], F32, tag="srow",
                                         name="srow")
                        nc.scalar.activation(srow, ss, Sqrt,
                                             scale=1.0 / dim, bias=eps_t)
                        vrow = rmsp.tile([1, QSB], F32, tag="vrow",
                                         name="vrow")
                        nc.scalar.activation(vrow, ss, Ident,
                                             scale=1.0 / dim, bias=eps_t)
                        r0 = rmsp.tile([1, QSB], F32, tag="r0", name="r0")
                        nc.vector.reciprocal(r0, srow)
                        # Newton: r1 = r0*(1.5 - 0.5*v*r0^2)
                        t1 = rmsp.tile([1, QSB], F32, tag="t1", name="t1")
                        nc.vector.tensor_mul(t1, r0, r0)
                        nc.vector.tensor_mul(t1, t1, vrow)
                        nc.vector.tensor_scalar(t1, t1, -0.5, 1.5, MUL, ADD)
                        nc.vector.tensor_mul(t1, t1, r0)
                        nc.sync.dma_start(out=rs_scr[side:side + 1, qs],
                                          in_=t1)


            # rope on k (headless): krot_raw rows lo 0:32, hi 32:64
            with tc.tile_pool(name="kr", bufs=1) as krp:
                ka1 = krp.tile([32, S], F32)
                ka2 = krp.tile([32, S], F32)
                nc.vector.tensor_mul(ka1, krot_raw[0:32], coslo[0:32])
                nc.vector.tensor_mul(ka2, krot_raw[32:64], sinlo[32:64])
                nc.vector.tensor_sub(krope[0:32], ka1, ka2)
                nc.vector.tensor_mul(ka1, krot_raw[32:64], coshi[32:64])
                nc.vector.tensor_mul(ka2, krot_raw[0:32], sinhi[0:32])
                nc.vector.tensor_add(krope[32:64], ka1, ka2)

        # ---------------- Phase C: per-head-pair q_b/kv_b/attn ----------
        with (
            tc.tile_pool(name="scales", bufs=1) as scp,
            tc.tile_pool(name="pairq", bufs=1) as pairq,
            tc.tile_pool(name="pairw", bufs=2) as pairw,
            tc.tile_pool(name="ropet", bufs=1) as ropet,
            tc.tile_pool(name="exps", bufs=4) as expp,
            tc.tile_pool(name="dn", bufs=2) as dnp,
            tc.tile_pool(name="ps_pj", bufs=1, space="PSUM") as pspj,
            tc.tile_pool(name="ps_sc", bufs=2, space="PSUM") as pssc,
            tc.tile_pool(name="ps_av", bufs=2, space="PSUM") as psav,
            tc.tile_pool(name="ps_dn", bufs=1, space="PSUM") as psdn,
        ):
            Rq = scp.tile([128, S], F32)
            nc.sync.dma_start(out=Rq, in_=rs_scr[0:1].to_broadcast([128, S]))
            Rkv = scp.tile([128, S], F32)
            nc.sync.dma_start(out=Rkv, in_=rs_scr[1:2].to_broadcast([128, S]))
            rkv_c = scp.tile([128, NKB, 1], F32)
            rs_colT = rs_scr[1:2].rearrange("o (kb p) -> kb p o", p=128)
            for kb in range(NKB):
                nc.sync.dma_start(out=rkv_c[:, kb, :], in_=rs_colT[kb])
            for hp2 in range(4):
                # --- q_b for the pair: m-tiles [nope0, nope1, lohi] ---
                qT_nope = [pairq.tile([128, S], F32R, tag=f"qTn{m}",
                                      name=f"qTn{m}") for m in range(2)]
                qlohi = pairq.tile([128, S], F32, tag="qlohi", name="qlohi")
                for i2 in range(NQSB):
                    qs = slice(i2 * QSB, (i2 + 1) * QSB)
                    pss = [pspj.tile([128, QSB], F32, tag=f"p{mi}",
                                     name=f"ps_qb{mi}") for mi in range(3)]
                    for r in range(NRQ):
                        qb_t = pairw.tile([128, 384], F32R, tag="qbw",
                                          name="qb_t")
                        nc.sync.dma_start(
                            out=qb_t,
                            in_=qbw_d[r * 128:(r + 1) * 128,
                                      hp2 * 384:(hp2 + 1) * 384])
                        for mi in range(3):
                            nc.tensor.matmul(
                                pss[mi], qb_t[:, mi * 128:(mi + 1) * 128],
                                cq_raw[r][:, qs],
                                start=(r == 0), stop=(r == NRQ - 1))
                    nc.vector.tensor_mul(qT_nope[0][:, qs], pss[0],
                                         Rq[:, qs])
                    nc.vector.tensor_mul(qT_nope[1][:, qs], pss[1],
                                         Rq[:, qs])
                    nc.vector.tensor_mul(qlohi[:, qs], pss[2], Rq[:, qs])
                # --- rope on q pair ---
                # qlohi rows: lo h0 0:32 | lo h1 32:64 | hi h0 64:96 | hi h1
                qrope = pairq.tile([64, 2, S], F32R, tag="qrope",
                                   name="qrope")
                a1 = ropet.tile([64, S], F32, tag="a1", name="a1")
                a2 = ropet.tile([64, S], F32, tag="a2", name="a2")
                nc.vector.tensor_mul(a1, qlohi[0:64], coslo[0:64])
                nc.vector.tensor_mul(a2, qlohi[64:128], sinlo[64:128])
                for hh in range(2):
                    nc.vector.tensor_sub(qrope[0:32, hh, :],
                                         a1[hh * 32:(hh + 1) * 32, :],
                                         a2[hh * 32:(hh + 1) * 32, :])
                nc.vector.tensor_mul(a1, qlohi[64:128], coshi[64:128])
                nc.vector.tensor_mul(a2, qlohi[0:64], sinhi[0:64])
                for hh in range(2):
                    nc.vector.tensor_add(qrope[32:64, hh, :],
                                         a1[hh * 32:(hh + 1) * 32, :],
                                         a2[hh * 32:(hh + 1) * 32, :])

                # --- kv_b for the pair ---
                kvn_t = pairw.tile([128, NRKV, 256], F32R, tag="kvn",
                                   name="kvn_t")
                kvv_t = pairw.tile([128, NRKV, 256], F32R, tag="kvv",
                                   name="kvv_t")
                for r in range(NRKV):
                    nc.sync.dma_start(
                        out=kvn_t[:, r, :],
                        in_=kvbn_d[r * 128:(r + 1) * 128,
                                   hp2 * 256:(hp2 + 1) * 256])
                    nc.sync.dma_start(
                        out=kvv_t[:, r, :],
                        in_=kvbv_d[r * 128:(r + 1) * 128,
                                   hp2 * 256:(hp2 + 1) * 256])
                k_passT = [pairq.tile([128, S], F32R, tag=f"kT{m}",
                                      name=f"kT{m}") for m in range(2)]
                for i2 in range(NQSB):
                    qs = slice(i2 * QSB, (i2 + 1) * QSB)
                    pk = [pspj.tile([128, QSB], F32, tag=f"p{mi}",
                                    name=f"ps_kv{mi}") for mi in range(2)]
                    for r in range(NRKV):
                        for mi in range(2):
                            nc.tensor.matmul(
                                pk[mi],
                                kvn_t[:, r, mi * 128:(mi + 1) * 128],
                                ckv_raw[r][:, qs],
                                start=(r == 0), stop=(r == NRKV - 1))
                    for mi in range(2):
                        nc.vector.tensor_mul(k_passT[mi][:, qs], pk[mi],
                                             Rkv[:, qs])
                v_p = pairq.tile([128, NKB, 2 * VH], F32R, tag="vp",
                                 name="v_p")
                for kb in range(NKB):
                    psv = pspj.tile([128, 2 * VH], F32, tag="p2", name="psv")
                    for r in range(NRKV):
                        nc.tensor.matmul(
                            psv, ckv_raw[r][:, kb * 128:(kb + 1) * 128],
                            kvv_t[:, r, :],
                            start=(r == 0), stop=(r == NRKV - 1))
                    nc.vector.tensor_scalar_mul(v_p[:, kb, :], psv,
                                                rkv_c[:, kb, :])

                # --- attention for both heads of the pair ---
                for hh in range(2):
                    h = hp2 * 2 + hh
                    for i2 in range(NQSB):
                        qs = slice(i2 * QSB, (i2 + 1) * QSB)
                        js = [j for j in range(NKB)
                              if plan[i2][j] != 'skip']
                        ets = []
                        for j in js:
                            ps = pssc.tile([128, QSB], F32, tag="sc",
                                           name="ps_sc")
                            nc.tensor.matmul(
                                ps,
                                k_passT[hh][:, j * 128:(j + 1) * 128],
                                qT_nope[hh][:, qs], start=True, stop=False)
                            nc.tensor.matmul(
                                ps, krope[:, j * 128:(j + 1) * 128],
                                qrope[:, hh, qs], start=False, stop=True)
                            pat = plan[i2][j]
                            if pat is not None:
                                nc.vector.tensor_add(ps, ps,
                                                     maskp[:, pat, :])
                            et = expp.tile([128, QSB], F32R, tag="exp",
                                           name="et")
                            nc.scalar.activation(et, ps, Exp, scale=SCALING)
                            ets.append(et)
                        pa = psav.tile([128, QSB], F32, tag="av", name="pa")
                        pd = psdn.tile([1, QSB], F32, tag="dn", name="pd")
                        for n, (j, et) in enumerate(zip(js, ets)):
                            nc.tensor.matmul(
                                pa, v_p[:, j, hh * 128:hh * 128 + 128], et,
                                start=(n == 0), stop=(n == len(js) - 1))
                            nc.tensor.matmul(
                                pd, ones_r, et,
                                start=(n == 0), stop=(n == len(js) - 1))
                        rec = dnp.tile([1, QSB], F32, tag="rec", name="rec")
                        nc.vector.reciprocal(rec, pd)
                        sl = h * NQSB + i2
                        nc.sync.dma_start(out=den_scr[sl:sl + 1, :], in_=rec)
                        bc = dnp.tile([128, QSB], F32, tag="bc", name="bc")
                        nc.sync.dma_start(
                            out=bc,
                            in_=den_scr[sl:sl + 1, :].to_broadcast(
                                [128, QSB]))
                        ao = dnp.tile([128, QSB], F32R, tag="ao", name="ao")
                        nc.vector.tensor_mul(ao, pa, bc)
                        nc.sync.dma_start(out=attn_scr[h][:, qs], in_=ao)

        # ---------------- Phase D: o_proj ----------------
        with (
            tc.tile_pool(name="ow", bufs=1) as owp,
            tc.tile_pool(name="ats", bufs=3) as atsp,
            tc.tile_pool(name="ob", bufs=4) as obp,
            tc.tile_pool(name="ps_o", bufs=4, space="PSUM") as pso,
        ):
            EH = HID // 2  # 2048 per half
            for eh in range(2):
                ow_sb = owp.tile([128, HG, EH], F32R, tag="ow", name="ow_sb")
                for c in range(HG):
                    nc.sync.dma_start(
                        out=ow_sb[:, c, :],
                        in_=ow_d[c * 128:(c + 1) * 128,
                                 eh * EH:(eh + 1) * EH])
                for i in range(NKB):
                    at_i = atsp.tile([128, HG, 128], F32R, tag="at",
                                     name="at_i")
                    for c in range(HG):
                        nc.sync.dma_start(
                            out=at_i[:, c, :],
                            in_=attn_scr[c][:, i * 128:(i + 1) * 128])
                    for es in range(EH // QSB):
                        po = pso.tile([128, QSB], F32, tag="po", name="po")
                        for c in range(HG):
                            nc.tensor.matmul(
                                po, at_i[:, c, :],
                                ow_sb[:, c, es * QSB:(es + 1) * QSB],
                                start=(c == 0), stop=(c == HG - 1))
                        ob = obp.tile([128, QSB], F32, tag="ob", name="ob")
                        nc.scalar.copy(ob, po)
                        nc.sync.dma_start(
                            out=out_d[i * 128:(i + 1) * 128,
                                      eh * EH + es * QSB:
                                      eh * EH + (es + 1) * QSB],
                            in_=ob)
    nc.compile()
    return nc


def _prep_inputs(hidden_states, cos, sin, attention_mask, q_a_w, q_a_ln_w,
                 q_b_w, kv_a_w, kv_a_ln_w, kv_b_w, o_w):
    """Build the 8 per-core input maps + the mask plan."""
    f = np.float32
    plan, patterns = _mask_plan(np.asarray(attention_mask, f))

    wa = np.ascontiguousarray(
        np.concatenate([np.asarray(q_a_w, f), np.asarray(kv_a_w, f)], axis=1))
    qbw_full = np.asarray(q_a_ln_w, f)[:, None] * np.asarray(q_b_w, f)
    kvb_full = np.asarray(kv_a_ln_w, f)[:, None] * np.asarray(kv_b_w, f)
    kvb_full = kvb_full.reshape(KVLR, H, NOPE + VH)
    o_w = np.asarray(o_w, f)

    per_batch = []
    for b in range(B):
        hT = np.ascontiguousarray(np.asarray(hidden_states, f)[b].T)
        cosT = np.asarray(cos, f)[b].T  # [ROPE, S]
        sinT = np.asarray(sin, f)[b].T
        per_batch.append(dict(
            hT=hT,
            coslo=np.ascontiguousarray(np.tile(cosT[:32], (4, 1))),
            coshi=np.ascontiguousarray(np.tile(cosT[32:], (4, 1))),
            sinlo=np.ascontiguousarray(np.tile(sinT[:32], (4, 1))),
            sinhi=np.ascontiguousarray(np.tile(sinT[32:], (4, 1))),
        ))

    in_maps = []
    for c in range(NCORES):
        b, g = divmod(c, 4)
        hs = g * HG  # first head of this core
        qb = qbw_full[:, hs * QKH:(hs + HG) * QKH].reshape(QLR, HG, QKH)
        cols = []
        for p in range(4):  # head pairs
            h0, h1 = 2 * p, 2 * p + 1
            cols += [qb[:, h0, :NOPE], qb[:, h1, :NOPE],
                     qb[:, h0, NOPE:NOPE + 32], qb[:, h1, NOPE:NOPE + 32],
                     qb[:, h0, NOPE + 32:], qb[:, h1, NOPE + 32:]]
        qbw = np.ascontiguousarray(np.concatenate(cols, axis=1))
        kvb = kvb_full[:, hs:hs + HG]
        kvbn = np.ascontiguousarray(kvb[:, :, :NOPE].reshape(KVLR, HG * NOPE))
        kvbv = np.ascontiguousarray(kvb[:, :, NOPE:].reshape(KVLR, HG * VH))
        ow = np.ascontiguousarray(o_w[hs * VH:(hs + HG) * VH])
        m = dict(per_batch[b])
        m.update(wa=wa, qbw=qbw, kvbn=kvbn, kvbv=kvbv, ow=ow,
                 ones=np.ones((128, 1), f))
        if patterns:
            m["maskpat"] = np.ascontiguousarray(np.stack(patterns))
        in_maps.append(m)
    return in_maps, plan, patterns


def kernel(**inputs):
    from concourse import bass_utils

    in_maps, plan, patterns = _prep_inputs(**inputs)
    key = str(plan)
    if key not in _CACHE:
        _CACHE[key] = _build(plan, len(patterns))
    nc = _CACHE[key]
    res = bass_utils.run_bass_kernel_spmd(nc, in_maps,
                                          core_ids=list(range(NCORES)))
    out = np.zeros((B, S, HID), np.float32)
    for c in range(NCORES):
        out[c // 4] += res.results[c]["outp"]
    return out


# revision 8
# speedup vs baseline: 1.0945x; 1.0945x over previous
"""DeepseekV3 MLA attention on 8 trn2 NeuronCores.

Sharding: data-parallel over batch (2 groups of 4 cores) x tensor-parallel
over heads (4-way, 8 heads/core). core c handles batch c//4, heads
(c%4)*8..+8. The latent stems (q_a / kv_a projections) are column-sharded
across the 4 cores of each batch group and reassembled with an in-group
AllGather (the per-token partial sums-of-squares for RMSNorm ride along in
the same buffer). Each core then computes its head-slice of q_b/kv_b and
attention, and a partial o_proj output; the host sums the 4 partials.

Everything on-device lives in transposed [dim, token] layout so no PE
transposes are needed anywhere:
  stem:   c_qT = wa^T @ hiddenT        (lhsT=wa chunk, rhs=hiddenT chunk)
  q_b:    qT = qbw^T @ c_qT
  scores: sT[k,q]: lhsT=kT[d,kblk], rhs=qT[d,qsb]   (PSUM [k,q])
  AV:     outT[vh,q]: lhsT=v[k,vh], rhs=expT[k,q]
  o_proj: o[q,e]: lhsT=attn_outT[c,qblk], rhs=ow[c,e]
The RMS scales are folded in at the q_b/kv_b copybacks (per-token scaling
commutes through the latent-dim contraction). Softmax skips the
max-subtraction (scores are O(1) here; masked lanes underflow exp to 0);
the denominator is accumulated with a ones-column matmul and applied as
exp(-ln(den)) to avoid slow single-lane reciprocals. All matmuls run in
float32r (FP22-truncated fp32), which streams at bf16 rate for moving
free dims >= 256.
"""

import numpy as np

B, S, HID = 2, 1024, 4096
H, NOPE, ROPE, QKH, VH = 32, 128, 64, 192, 128
QLR, KVLR = 1536, 512
EPS = 1e-6
SCALING = QKH ** -0.5
HG = 8            # heads per core
NCORES = 8
QSB = 512         # q superblock (matmul moving free dim)
NQSB = S // QSB   # 2
KB = 128          # k block
NKB = S // KB     # 8
NEG_SKIP = -1e8   # mask values below this => exp == 0 => block skipped
SHW = 576         # stem shard width: 3 q-tiles + 1 kv-tile + 64 (krot/pad)
AGR = SHW + 2     # shard rows + 2 partial-sumsq rows

_CACHE = {}


def _mask_plan(mask):
    """Classify each (qsb, kblock) of the additive mask.

    plan[i2][j] is None (no mask), 'skip' (fully masked), or an index into
    patterns (list of [128, QSB] f32 blocks holding maskT/SCALING).
    """
    maskT = np.ascontiguousarray(mask.reshape(S, S).T)  # [k, q]
    patterns = []
    keys = {}
    plan = []
    for i2 in range(NQSB):
        row = []
        for j in range(NKB):
            blk = maskT[j * KB:(j + 1) * KB, i2 * QSB:(i2 + 1) * QSB]
            if np.all(blk == 0.0):
                row.append(None)
            elif np.all(blk <= NEG_SKIP):
                row.append('skip')
            else:
                key = blk.tobytes()
                if key not in keys:
                    keys[key] = len(patterns)
                    patterns.append((blk / SCALING).astype(np.float32))
                row.append(keys[key])
        plan.append(row)
    return plan, patterns


def _build(plan, n_pat):
    import concourse.mybir as mybir
    import concourse.tile as tile
    from concourse import bacc

    F32 = mybir.dt.float32
    F32R = mybir.dt.float32r
    Exp = mybir.ActivationFunctionType.Exp
    Ln = mybir.ActivationFunctionType.Ln
    Sqrt = mybir.ActivationFunctionType.Sqrt
    Ident = mybir.ActivationFunctionType.Identity
    Square = mybir.ActivationFunctionType.Square
    Copy = mybir.ActivationFunctionType.Copy
    MUL = mybir.AluOpType.mult
    ADD = mybir.AluOpType.add

    nc = bacc.Bacc("TRN2", target_bir_lowering=False, debug=False,
                   enable_asserts=False, num_devices=NCORES)

    hT_d = nc.dram_tensor("hT", [HID, S], F32R, kind="ExternalInput").ap()
    # per-core stem shard weights: [3 q-tiles | kv-tile | krot-or-pad]
    wa_d = nc.dram_tensor("wa", [HID, SHW], F32R, kind="ExternalInput").ap()
    # qbw: per pair p: [nope h0 |nope h1 |lo h0 |lo h1 |hi h0 |hi h1] = 384
    qbw_d = nc.dram_tensor("qbw", [QLR, HG * QKH], F32R,
                           kind="ExternalInput").ap()
    kvbn_d = nc.dram_tensor("kvbn", [KVLR, HG * NOPE], F32R,
                            kind="ExternalInput").ap()
    kvbv_d = nc.dram_tensor("kvbv", [KVLR, HG * VH], F32R,
                            kind="ExternalInput").ap()
    ow_d = nc.dram_tensor("ow", [HG * VH, HID], F32R,
                          kind="ExternalInput").ap()
    # trig tables 4x row-tiled so any 32-aligned slice has a matching base
    coslo_d = nc.dram_tensor("coslo", [128, S], F32, kind="ExternalInput").ap()
    coshi_d = nc.dram_tensor("coshi", [128, S], F32, kind="ExternalInput").ap()
    sinlo_d = nc.dram_tensor("sinlo", [128, S], F32, kind="ExternalInput").ap()
    sinhi_d = nc.dram_tensor("sinhi", [128, S], F32, kind="ExternalInput").ap()
    ones_d = nc.dram_tensor("ones", [128, 1], F32R, kind="ExternalInput").ap()
    if n_pat:
        mp_d = nc.dram_tensor("maskpat", [n_pat, 128, QSB], F32,
                              kind="ExternalInput").ap()
    out_d = nc.dram_tensor("outp", [S, HID], F32, kind="ExternalOutput").ap()

    ag_in = nc.dram_tensor("ag_in", [AGR, S], F32R, kind="Internal").ap()
    ag_out = nc.dram_tensor("ag_out", [4, AGR, S], F32R, kind="Internal").ap()
    rs_scr = nc.dram_tensor("rs_scr", [2, S], F32, kind="Internal").ap()
    den_scr = nc.dram_tensor("den_scr", [HG * NQSB, QSB], F32,
                             kind="Internal").ap()
    attn_scr = nc.dram_tensor("attn_scr", [HG, 128, S], F32R,
                              kind="Internal").ap()

    NE = HID // 128      # 32 stem contraction chunks
    NRQ = QLR // 128     # 12
    NRKV = KVLR // 128   # 4
    SH_W = [128, 128, 128, 128, 64]   # local stem m-tile widths

    hT_re = hT_d.rearrange("(e p) q -> p e q", p=128)

    with nc.allow_low_precision(reason="float32r tiles carry fp32 bits"), \
         tile.TileContext(nc) as tc:
      with tc.tile_pool(name="singles", bufs=1) as singles, \
           tc.tile_pool(name="lat", bufs=1) as latp:
        ones_r = singles.tile([128, 1], F32R)
        nc.sync.dma_start(out=ones_r, in_=ones_d)
        eps_t = singles.tile([1, 1], F32)
        nc.vector.memset(eps_t, EPS)
        coslo = singles.tile([128, S], F32)
        coshi = singles.tile([128, S], F32)
        sinlo = singles.tile([128, S], F32)
        sinhi = singles.tile([128, S], F32)
        nc.sync.dma_start(out=coslo, in_=coslo_d)
        nc.sync.dma_start(out=coshi, in_=coshi_d)
        nc.sync.dma_start(out=sinlo, in_=sinlo_d)
        nc.sync.dma_start(out=sinhi, in_=sinhi_d)
        if n_pat:
            maskp = singles.tile([128, n_pat, QSB], F32)
            for p in range(n_pat):
                nc.sync.dma_start(out=maskp[:, p, :], in_=mp_d[p])

        # full latents (assembled post-AllGather)
        cq_raw = [latp.tile([128, S], F32R, name=f"cq_raw{r}")
                  for r in range(NRQ)]
        ckv_raw = [latp.tile([128, S], F32R, name=f"ckv_raw{r}")
                   for r in range(NRKV)]
        krot_raw = latp.tile([64, S], F32R)
        krope = latp.tile([64, S], F32R)

        # ---------------- Phase A: sharded stem ----------------
        with (
            tc.tile_pool(name="shard", bufs=1) as shp,
            tc.tile_pool(name="stem_h", bufs=1) as hp,
            tc.tile_pool(name="stem_w", bufs=4) as wap,
            tc.tile_pool(name="sq", bufs=3) as sqp,
            tc.tile_pool(name="ssr", bufs=1) as ssrp,
            tc.tile_pool(name="stem_ps", bufs=1, space="PSUM") as psst,
        ):
            sh = [shp.tile([SH_W[t], S], F32R, name=f"sh{t}")
                  for t in range(5)]
            ssq_sb = ssrp.tile([1, S], F32R)
            sskv_sb = ssrp.tile([1, S], F32R)
            for i2 in range(NQSB):
                qs = slice(i2 * QSB, (i2 + 1) * QSB)
                hT_sb = hp.tile([128, NE, QSB], F32R, tag="ht", name="hT_sb")
                for e in range(NE):
                    nc.sync.dma_start(out=hT_sb[:, e, :], in_=hT_re[:, e, qs])
                pss = [psst.tile([SH_W[t], QSB], F32, name=f"ps_stem{t}",
                                 tag=f"s{t}") for t in range(5)]
                for e in range(NE):
                    wa_t = wap.tile([128, SHW], F32R, tag="wa", name="wa_t")
                    nc.sync.dma_start(
                        out=wa_t, in_=wa_d[e * 128:(e + 1) * 128, :])
                    for t in range(5):
                        nc.tensor.matmul(
                            pss[t], wa_t[:, t * 128:t * 128 + SH_W[t]],
                            hT_sb[:, e, :],
                            start=(e == 0), stop=(e == NE - 1))
                for t in range(5):
                    nc.vector.tensor_copy(sh[t][:, qs], pss[t])
                # local partial sum-of-squares (q: tiles 0-2, kv: tile 3)
                ssq = psst.tile([1, QSB], F32, tag="ssq", name="ssq")
                for t in range(3):
                    sq = sqp.tile([128, QSB], F32R, tag="sq", name="sq")
                    nc.scalar.activation(sq, sh[t][:, qs], Square)
                    nc.tensor.matmul(ssq, ones_r, sq,
                                     start=(t == 0), stop=(t == 2))
                sskv = psst.tile([1, QSB], F32, tag="sskv", name="sskv")
                sq3 = sqp.tile([128, QSB], F32R, tag="sq", name="sq3")
                nc.scalar.activation(sq3, sh[3][:, qs], Square)
                nc.tensor.matmul(sskv, ones_r, sq3, start=True, stop=True)
                nc.scalar.activation(ssq_sb[:, qs], ssq, Copy)
                nc.scalar.activation(sskv_sb[:, qs], sskv, Copy)
            # ship shard + partials to the AllGather buffer
            for t in range(5):
                nc.sync.dma_start(out=ag_in[t * 128:t * 128 + SH_W[t], :],
                                  in_=sh[t])
            nc.sync.dma_start(out=ag_in[SHW:SHW + 1, :], in_=ssq_sb)
            nc.sync.dma_start(out=ag_in[SHW + 1:SHW + 2, :], in_=sskv_sb)

        nc.gpsimd.collective_compute(
            "AllGather", mybir.AluOpType.bypass,
            replica_groups=[[0, 1, 2, 3], [4, 5, 6, 7]],
            ins=[ag_in], outs=[ag_out])

        # ---------------- Phase B: reassemble + rmsnorm rows ----------
        with (
            tc.tile_pool(name="rms", bufs=4) as rmsp,
            tc.tile_pool(name="rms_ps", bufs=2, space="PSUM") as psss,
        ):
            for r in range(NRQ):
                g, lt = divmod(r, 3)
                nc.sync.dma_start(
                    out=cq_raw[r],
                    in_=ag_out[g, lt * 128:(lt + 1) * 128, :])
            for r in range(NRKV):
                nc.sync.dma_start(out=ckv_raw[r],
                                  in_=ag_out[r, 384:512, :])
            nc.sync.dma_start(out=krot_raw, in_=ag_out[0, 512:576, :])
            ssg = rmsp.tile([4, S], F32R, tag="ssg", name="ssg")
            sskvg = rmsp.tile([4, S], F32R, tag="sskvg", name="sskvg")
            nc.sync.dma_start(out=ssg, in_=ag_out[:, SHW, :])
            nc.sync.dma_start(out=sskvg, in_=ag_out[:, SHW + 1, :])
            for side, src, dim in ((0, ssg, QLR), (1, sskvg, KVLR)):
                for i2 in range(NQSB):
                    qs = slice(i2 * QSB, (i2 + 1) * QSB)
                    ss = psss.tile([1, QSB], F32, tag="ss", name="ss")
                    nc.tensor.matmul(ss, ones_r[0:4, :], src[:, qs],
                                     start=True, stop=True)
                    srow = rmsp.tile([1, QSB], F32, tag="srow", name="srow")
                    nc.scalar.activation(srow, ss, Sqrt,
                                         scale=1.0 / dim, bias=eps_t)
                    vrow = rmsp.tile([1, QSB], F32, tag="vrow", name="vrow")
                    nc.scalar.activation(vrow, ss, Ident,
                                         scale=1.0 / dim, bias=eps_t)
                    r0 = rmsp.tile([1, QSB], F32, tag="r0", name="r0")
                    nc.vector.reciprocal(r0, srow)
                    # Newton: r1 = r0*(1.5 - 0.5*v*r0^2)
                    t1 = rmsp.tile([1, QSB], F32, tag="t1", name="t1")
                    nc.vector.tensor_mul(t1, r0, r0)
                    nc.vector.tensor_mul(t1, t1, vrow)
                    nc.vector.tensor_scalar(t1, t1, -0.5, 1.5, MUL, ADD)
                    nc.vector.tensor_mul(t1, t1, r0)
                    nc.sync.dma_start(out=rs_scr[side:side + 1, qs], in_=t1)

        # rope on k (headless): krot_raw rows lo 0:32, hi 32:64
        with tc.tile_pool(name="kr", bufs=1) as krp:
            ka1 = krp.tile([32, S], F32)
            ka2 = krp.tile([32, S], F32)
            nc.vector.tensor_mul(ka1, krot_raw[0:32], coslo[0:32])
            nc.vector.tensor_mul(ka2, krot_raw[32:64], sinlo[32:64])
            nc.vector.tensor_sub(krope[0:32], ka1, ka2)
            nc.vector.tensor_mul(ka1, krot_raw[32:64], coshi[32:64])
            nc.vector.tensor_mul(ka2, krot_raw[0:32], sinhi[0:32])
            nc.vector.tensor_add(krope[32:64], ka1, ka2)

        # ---------------- Phase C: per-head-pair q_b/kv_b/attn ----------
        with (
            tc.tile_pool(name="scales", bufs=1) as scp,
            tc.tile_pool(name="pairq", bufs=1) as pairq,
            tc.tile_pool(name="pairw", bufs=2) as pairw,
            tc.tile_pool(name="ropet", bufs=1) as ropet,
            tc.tile_pool(name="exps", bufs=4) as expp,
            tc.tile_pool(name="dn", bufs=2) as dnp,
            tc.tile_pool(name="ps_pj", bufs=1, space="PSUM") as pspj,
            tc.tile_pool(name="ps_sc", bufs=2, space="PSUM") as pssc,
            tc.tile_pool(name="ps_av", bufs=2, space="PSUM") as psav,
            tc.tile_pool(name="ps_dn", bufs=1, space="PSUM") as psdn,
        ):
            Rq = scp.tile([128, S], F32)
            nc.sync.dma_start(out=Rq, in_=rs_scr[0:1].to_broadcast([128, S]))
            Rkv = scp.tile([128, S], F32)
            nc.sync.dma_start(out=Rkv, in_=rs_scr[1:2].to_broadcast([128, S]))
            rkv_c = scp.tile([128, NKB, 1], F32)
            rs_colT = rs_scr[1:2].rearrange("o (kb p) -> kb p o", p=128)
            for kb in range(NKB):
                nc.sync.dma_start(out=rkv_c[:, kb, :], in_=rs_colT[kb])
            for hp2 in range(4):
                # --- q_b for the pair: m-tiles [nope0, nope1, lohi] ---
                qT_nope = [pairq.tile([128, S], F32R, tag=f"qTn{m}",
                                      name=f"qTn{m}") for m in range(2)]
                qlohi = pairq.tile([128, S], F32, tag="qlohi", name="qlohi")
                for i2 in range(NQSB):
                    qs = slice(i2 * QSB, (i2 + 1) * QSB)
                    pss = [pspj.tile([128, QSB], F32, tag=f"p{mi}",
                                     name=f"ps_qb{mi}") for mi in range(3)]
                    for r in range(NRQ):
                        qb_t = pairw.tile([128, 384], F32R, tag="qbw",
                                          name="qb_t")
                        nc.sync.dma_start(
                            out=qb_t,
                            in_=qbw_d[r * 128:(r + 1) * 128,
                                      hp2 * 384:(hp2 + 1) * 384])
                        for mi in range(3):
                            nc.tensor.matmul(
                                pss[mi], qb_t[:, mi * 128:(mi + 1) * 128],
                                cq_raw[r][:, qs],
                                start=(r == 0), stop=(r == NRQ - 1))
                    nc.vector.tensor_mul(qT_nope[0][:, qs], pss[0],
                                         Rq[:, qs])
                    nc.vector.tensor_mul(qT_nope[1][:, qs], pss[1],
                                         Rq[:, qs])
                    nc.vector.tensor_mul(qlohi[:, qs], pss[2], Rq[:, qs])
                # --- rope on q pair ---
                # qlohi rows: lo h0 0:32 | lo h1 32:64 | hi h0 64:96 | hi h1
                qrope = pairq.tile([64, 2, S], F32R, tag="qrope",
                                   name="qrope")
                a1 = ropet.tile([64, S], F32, tag="a1", name="a1")
                a2 = ropet.tile([64, S], F32, tag="a2", name="a2")
                nc.vector.tensor_mul(a1, qlohi[0:64], coslo[0:64])
                nc.vector.tensor_mul(a2, qlohi[64:128], sinlo[64:128])
                for hh in range(2):
                    nc.vector.tensor_sub(qrope[0:32, hh, :],
                                         a1[hh * 32:(hh + 1) * 32, :],
                                         a2[hh * 32:(hh + 1) * 32, :])
                nc.vector.tensor_mul(a1, qlohi[64:128], coshi[64:128])
                nc.vector.tensor_mul(a2, qlohi[0:64], sinhi[0:64])
                for hh in range(2):
                    nc.vector.tensor_add(qrope[32:64, hh, :],
                                         a1[hh * 32:(hh + 1) * 32, :],
                                         a2[hh * 32:(hh + 1) * 32, :])

                # --- kv_b for the pair ---
                kvn_t = pairw.tile([128, NRKV, 256], F32R, tag="kvn",
                                   name="kvn_t")
                kvv_t = pairw.tile([128, NRKV, 256], F32R, tag="kvv",
                                   name="kvv_t")
                for r in range(NRKV):
                    nc.sync.dma_start(
                        out=kvn_t[:, r, :],
                        in_=kvbn_d[r * 128:(r + 1) * 128,
                                   hp2 * 256:(hp2 + 1) * 256])
                    nc.sync.dma_start(
                        out=kvv_t[:, r, :],
                        in_=kvbv_d[r * 128:(r + 1) * 128,
                                   hp2 * 256:(hp2 + 1) * 256])
                k_passT = [pairq.tile([128, S], F32R, tag=f"kT{m}",
                                      name=f"kT{m}") for m in range(2)]
                for i2 in range(NQSB):
                    qs = slice(i2 * QSB, (i2 + 1) * QSB)
                    pk = [pspj.tile([128, QSB], F32, tag=f"p{mi}",
                                    name=f"ps_kv{mi}") for mi in range(2)]
                    for r in range(NRKV):
                        for mi in range(2):
                            nc.tensor.matmul(
                                pk[mi],
                                kvn_t[:, r, mi * 128:(mi + 1) * 128],
                                ckv_raw[r][:, qs],
                                start=(r == 0), stop=(r == NRKV - 1))
                    for mi in range(2):
                        nc.vector.tensor_mul(k_passT[mi][:, qs], pk[mi],
                                             Rkv[:, qs])
                v_p = pairq.tile([128, NKB, 2 * VH], F32R, tag="vp",
                                 name="v_p")
                for kb in range(NKB):
                    psv = pspj.tile([128, 2 * VH], F32, tag="p2", name="psv")
                    for r in range(NRKV):
                        nc.tensor.matmul(
                            psv, ckv_raw[r][:, kb * 128:(kb + 1) * 128],
                            kvv_t[:, r, :],
                            start=(r == 0), stop=(r == NRKV - 1))
                    nc.vector.tensor_scalar_mul(v_p[:, kb, :], psv,
                                                rkv_c[:, kb, :])

                # --- attention for both heads of the pair ---
                for hh in range(2):
                    h = hp2 * 2 + hh
                    for i2 in range(NQSB):
                        qs = slice(i2 * QSB, (i2 + 1) * QSB)
                        js = [j for j in range(NKB)
                              if plan[i2][j] != 'skip']
                        ets = []
                        for j in js:
                            ps = pssc.tile([128, QSB], F32, tag="sc",
                                           name="ps_sc")
                            nc.tensor.matmul(
                                ps,
                                k_passT[hh][:, j * 128:(j + 1) * 128],
                                qT_nope[hh][:, qs], start=True, stop=False)
                            nc.tensor.matmul(
                                ps, krope[:, j * 128:(j + 1) * 128],
                                qrope[:, hh, qs], start=False, stop=True)
                            pat = plan[i2][j]
                            if pat is not None:
                                nc.vector.tensor_add(ps, ps,
                                                     maskp[:, pat, :])
                            et = expp.tile([128, QSB], F32R, tag="exp",
                                           name="et")
                            nc.scalar.activation(et, ps, Exp, scale=SCALING)
                            ets.append(et)
                        pa = psav.tile([128, QSB], F32, tag="av", name="pa")
                        pd = psdn.tile([1, QSB], F32, tag="dn", name="pd")
                        for n, (j, et) in enumerate(zip(js, ets)):
                            nc.tensor.matmul(
                                pa, v_p[:, j, hh * 128:hh * 128 + 128], et,
                                start=(n == 0), stop=(n == len(js) - 1))
                            nc.tensor.matmul(
                                pd, ones_r, et,
                                start=(n == 0), stop=(n == len(js) - 1))
                        lgd = dnp.tile([1, QSB], F32, tag="lgd", name="lgd")
                        nc.scalar.activation(lgd, pd, Ln)
                        sl = h * NQSB + i2
                        nc.sync.dma_start(out=den_scr[sl:sl + 1, :], in_=lgd)
                        bc = dnp.tile([128, QSB], F32, tag="bc", name="bc")
                        nc.sync.dma_start(
                            out=bc,
                            in_=den_scr[sl:sl + 1, :].to_broadcast(
                                [128, QSB]))
                        ne = dnp.tile([128, QSB], F32, tag="ne", name="ne")
                        nc.scalar.activation(ne, bc, Exp, scale=-1.0)
                        ao = dnp.tile([128, QSB], F32R, tag="ao", name="ao")
                        nc.vector.tensor_mul(ao, pa, ne)
                        nc.sync.dma_start(out=attn_scr[h][:, qs], in_=ao)

        # ---------------- Phase D: o_proj ----------------
        with (
            tc.tile_pool(name="ow", bufs=1) as owp,
            tc.tile_pool(name="ats", bufs=3) as atsp,
            tc.tile_pool(name="ob", bufs=4) as obp,
            tc.tile_pool(name="ps_o", bufs=4, space="PSUM") as pso,
        ):
            EH = HID // 2  # 2048 per half
            for eh in range(2):
                ow_sb = owp.tile([128, HG, EH], F32R, tag="ow", name="ow_sb")
                for c in range(HG):
                    nc.sync.dma_start(
                        out=ow_sb[:, c, :],
                        in_=ow_d[c * 128:(c + 1) * 128,
                                 eh * EH:(eh + 1) * EH])
                for i in range(NKB):
                    at_i = atsp.tile([128, HG, 128], F32R, tag="at",
                                     name="at_i")
                    for c in range(HG):
                        nc.sync.dma_start(
                            out=at_i[:, c, :],
                            in_=attn_scr[c][:, i * 128:(i + 1) * 128])
                    for es in range(EH // QSB):
                        po = pso.tile([128, QSB], F32, tag="po", name="po")
                        for c in range(HG):
                            nc.tensor.matmul(
                                po, at_i[:, c, :],
                                ow_sb[:, c, es * QSB:(es + 1) * QSB],
                                start=(c == 0), stop=(c == HG - 1))
                        ob = obp.tile([128, QSB], F32, tag="ob", name="ob")
                        nc.scalar.copy(ob, po)
                        nc.sync.dma_start(
                            out=out_d[i * 128:(i + 1) * 128,
                                      eh * EH + es * QSB:
                                      eh * EH + (es + 1) * QSB],
                            in_=ob)
    nc.compile()
    return nc


def _prep_inputs(hidden_states, cos, sin, attention_mask, q_a_w, q_a_ln_w,
                 q_b_w, kv_a_w, kv_a_ln_w, kv_b_w, o_w):
    """Build the 8 per-core input maps + the mask plan."""
    f = np.float32
    plan, patterns = _mask_plan(np.asarray(attention_mask, f))

    q_a_w = np.asarray(q_a_w, f)
    kv_a_w = np.asarray(kv_a_w, f)
    qbw_full = np.asarray(q_a_ln_w, f)[:, None] * np.asarray(q_b_w, f)
    kvb_full = np.asarray(kv_a_ln_w, f)[:, None] * np.asarray(kv_b_w, f)
    kvb_full = kvb_full.reshape(KVLR, H, NOPE + VH)
    o_w = np.asarray(o_w, f)

    per_batch = []
    for b in range(B):
        hT = np.ascontiguousarray(np.asarray(hidden_states, f)[b].T)
        cosT = np.asarray(cos, f)[b].T  # [ROPE, S]
        sinT = np.asarray(sin, f)[b].T
        per_batch.append(dict(
            hT=hT,
            coslo=np.ascontiguousarray(np.tile(cosT[:32], (4, 1))),
            coshi=np.ascontiguousarray(np.tile(cosT[32:], (4, 1))),
            sinlo=np.ascontiguousarray(np.tile(sinT[:32], (4, 1))),
            sinhi=np.ascontiguousarray(np.tile(sinT[32:], (4, 1))),
        ))

    in_maps = []
    for c in range(NCORES):
        b, g = divmod(c, 4)
        hs = g * HG  # first head of this core
        # stem shard weights: 3 q-tiles + 1 kv-tile + (krot | zeros)
        extra = (kv_a_w[:, KVLR:KVLR + ROPE] if g == 0
                 else np.zeros((HID, 64), f))
        wa = np.ascontiguousarray(np.concatenate(
            [q_a_w[:, 384 * g:384 * (g + 1)],
             kv_a_w[:, 128 * g:128 * (g + 1)], extra], axis=1))
        qb = qbw_full[:, hs * QKH:(hs + HG) * QKH].reshape(QLR, HG, QKH)
        cols = []
        for p in range(4):  # head pairs
            h0, h1 = 2 * p, 2 * p + 1
            cols += [qb[:, h0, :NOPE], qb[:, h1, :NOPE],
                     qb[:, h0, NOPE:NOPE + 32], qb[:, h1, NOPE:NOPE + 32],
                     qb[:, h0, NOPE + 32:], qb[:, h1, NOPE + 32:]]
        qbw = np.ascontiguousarray(np.concatenate(cols, axis=1))
        kvb = kvb_full[:, hs:hs + HG]
        kvbn = np.ascontiguousarray(kvb[:, :, :NOPE].reshape(KVLR, HG * NOPE))
        kvbv = np.ascontiguousarray(kvb[:, :, NOPE:].reshape(KVLR, HG * VH))
        ow = np.ascontiguousarray(o_w[hs * VH:(hs + HG) * VH])
        m = dict(per_batch[b])
        m.update(wa=wa, qbw=qbw, kvbn=kvbn, kvbv=kvbv, ow=ow,
                 ones=np.ones((128, 1), f))
        if patterns:
            m["maskpat"] = np.ascontiguousarray(np.stack(patterns))
        in_maps.append(m)
    return in_maps, plan, patterns


def kernel(**inputs):
    from concourse import bass_utils

    in_maps, plan, patterns = _prep_inputs(**inputs)
    key = str(plan)
    if key not in _CACHE:
        _CACHE[key] = _build(plan, len(patterns))
    nc = _CACHE[key]
    res = bass_utils.run_bass_kernel_spmd(nc, in_maps,
                                          core_ids=list(range(NCORES)))
    out = np.zeros((B, S, HID), np.float32)
    for c in range(NCORES):
        out[c // 4] += res.results[c]["outp"]
    return out


# revision 11
# speedup vs baseline: 1.1305x; 1.0329x over previous
"""DeepseekV3 MLA attention on 8 trn2 NeuronCores.

Sharding: data-parallel over batch (2 groups of 4 cores) x tensor-parallel
over heads (4-way, 8 heads/core). core c handles batch c//4, heads
(c%4)*8..+8. The latent stems (q_a / kv_a projections) are column-sharded
across the 4 cores of each batch group and reassembled with an in-group
AllGather (the per-token partial sums-of-squares for RMSNorm ride along in
the same buffer). Each core then computes its head-slice of q_b/kv_b and
attention, and a partial o_proj output; the host sums the 4 partials.

Everything on-device lives in transposed [dim, token] layout so no PE
transposes are needed anywhere:
  stem:   c_qT = wa^T @ hiddenT        (lhsT=wa chunk, rhs=hiddenT chunk)
  q_b:    qT = qbw^T @ c_qT
  scores: sT[k,q]: lhsT=kT[d,kblk], rhs=qT[d,qsb]   (PSUM [k,q])
  AV:     outT[vh,q]: lhsT=v[k,vh], rhs=expT[k,q]
  o_proj: o[q,e]: lhsT=attn_outT[c,qblk], rhs=ow[c,e]
The RMS scales are folded in at the q_b/kv_b copybacks (per-token scaling
commutes through the latent-dim contraction). Softmax skips the
max-subtraction (scores are O(1) here; masked lanes underflow exp to 0);
the denominator is accumulated with a ones-column matmul and applied as
exp(-ln(den)) to avoid slow single-lane reciprocals. All matmuls run in
float32r (FP22-truncated fp32), which streams at bf16 rate for moving
free dims >= 256.
"""

import numpy as np

B, S, HID = 2, 1024, 4096
H, NOPE, ROPE, QKH, VH = 32, 128, 64, 192, 128
QLR, KVLR = 1536, 512
EPS = 1e-6
SCALING = QKH ** -0.5
HG = 8            # heads per core
NCORES = 8
QSB = 512         # q superblock (matmul moving free dim)
NQSB = S // QSB   # 2
KB = 128          # k block
NKB = S // KB     # 8
NEG_SKIP = -1e8   # mask values below this => exp == 0 => block skipped
SHW = 576         # stem shard width: 3 q-tiles + 1 kv-tile + 64 (krot/pad)
AGR = SHW + 2     # shard rows + 2 partial-sumsq rows

_CACHE = {}


def _mask_plan(mask):
    """Classify each (qsb, kblock) of the additive mask.

    plan[i2][j] is None (no mask), 'skip' (fully masked), or an index into
    patterns (list of [128, QSB] f32 blocks holding maskT/SCALING).
    """
    maskT = np.ascontiguousarray(mask.reshape(S, S).T)  # [k, q]
    patterns = []
    keys = {}
    plan = []
    for i2 in range(NQSB):
        row = []
        for j in range(NKB):
            blk = maskT[j * KB:(j + 1) * KB, i2 * QSB:(i2 + 1) * QSB]
            if np.all(blk == 0.0):
                row.append(None)
            elif np.all(blk <= NEG_SKIP):
                row.append('skip')
            else:
                key = blk.tobytes()
                if key not in keys:
                    keys[key] = len(patterns)
                    patterns.append((blk / SCALING).astype(np.float32))
                row.append(keys[key])
        plan.append(row)
    return plan, patterns


def _build(plan, n_pat):
    import concourse.mybir as mybir
    import concourse.tile as tile
    from concourse import bacc

    F32 = mybir.dt.float32
    F32R = mybir.dt.float32r
    Exp = mybir.ActivationFunctionType.Exp
    Ln = mybir.ActivationFunctionType.Ln
    Sqrt = mybir.ActivationFunctionType.Sqrt
    Ident = mybir.ActivationFunctionType.Identity
    Square = mybir.ActivationFunctionType.Square
    Copy = mybir.ActivationFunctionType.Copy
    MUL = mybir.AluOpType.mult
    ADD = mybir.AluOpType.add

    nc = bacc.Bacc("TRN2", target_bir_lowering=False, debug=False,
                   enable_asserts=False, num_devices=NCORES)

    hT_d = nc.dram_tensor("hT", [HID, S], F32R, kind="ExternalInput").ap()
    # per-core stem shard weights: [3 q-tiles | kv-tile | krot-or-pad]
    wa_d = nc.dram_tensor("wa", [HID, SHW], F32R, kind="ExternalInput").ap()
    # qbw: per pair p: [nope h0 |nope h1 |lo h0 |lo h1 |hi h0 |hi h1] = 384
    qbw_d = nc.dram_tensor("qbw", [QLR, HG * QKH], F32R,
                           kind="ExternalInput").ap()
    kvbn_d = nc.dram_tensor("kvbn", [KVLR, HG * NOPE], F32R,
                            kind="ExternalInput").ap()
    kvbv_d = nc.dram_tensor("kvbv", [KVLR, HG * VH], F32R,
                            kind="ExternalInput").ap()
    ow_d = nc.dram_tensor("ow", [HG * VH, HID], F32R,
                          kind="ExternalInput").ap()
    # trig tables 4x row-tiled so any 32-aligned slice has a matching base
    coslo_d = nc.dram_tensor("coslo", [128, S], F32, kind="ExternalInput").ap()
    coshi_d = nc.dram_tensor("coshi", [128, S], F32, kind="ExternalInput").ap()
    sinlo_d = nc.dram_tensor("sinlo", [128, S], F32, kind="ExternalInput").ap()
    sinhi_d = nc.dram_tensor("sinhi", [128, S], F32, kind="ExternalInput").ap()
    ones_d = nc.dram_tensor("ones", [128, 1], F32R, kind="ExternalInput").ap()
    if n_pat:
        mp_d = nc.dram_tensor("maskpat", [n_pat, 128, QSB], F32,
                              kind="ExternalInput").ap()
    out_d = nc.dram_tensor("outp", [S, HID], F32, kind="ExternalOutput").ap()

    ag_in = nc.dram_tensor("ag_in", [AGR, S], F32R, kind="Internal").ap()
    ag_out = nc.dram_tensor("ag_out", [4, AGR, S], F32R, kind="Internal").ap()
    rs_scr = nc.dram_tensor("rs_scr", [2, S], F32, kind="Internal").ap()
    den_scr = nc.dram_tensor("den_scr", [HG * NQSB, QSB], F32,
                             kind="Internal").ap()
    attn_scr = nc.dram_tensor("attn_scr", [HG, 128, S], F32R,
                              kind="Internal").ap()

    NE = HID // 128      # 32 stem contraction chunks
    NRQ = QLR // 128     # 12
    NRKV = KVLR // 128   # 4
    SH_W = [128, 128, 128, 128, 64]   # local stem m-tile widths

    hT_re = hT_d.rearrange("(e p) q -> p e q", p=128)

    with nc.allow_low_precision(reason="float32r tiles carry fp32 bits"), \
         tile.TileContext(nc) as tc:
      with tc.tile_pool(name="singles", bufs=1) as singles:
        ones_r = singles.tile([128, 1], F32R)
        nc.sync.dma_start(out=ones_r, in_=ones_d)
        eps_t = singles.tile([1, 1], F32)
        nc.vector.memset(eps_t, EPS)
        coslo = singles.tile([128, S], F32)
        coshi = singles.tile([128, S], F32)
        sinlo = singles.tile([128, S], F32)
        sinhi = singles.tile([128, S], F32)
        if n_pat:
            maskp = singles.tile([128, n_pat, QSB], F32)
        # latents + kv_b weights live from the stem through the pair loop
        _latp_cm = tc.tile_pool(name="lat", bufs=1)
        latp = _latp_cm.__enter__()
        kvn_all = latp.tile([128, NRKV, HG * NOPE], F32R)
        kvv_all = latp.tile([128, NRKV, HG * VH], F32R)

        # full latents (assembled post-AllGather)
        cq_raw = [latp.tile([128, S], F32R, name=f"cq_raw{r}")
                  for r in range(NRQ)]
        ckv_raw = [latp.tile([128, S], F32R, name=f"ckv_raw{r}")
                   for r in range(NRKV)]
        krot_raw = latp.tile([64, S], F32R)
        krope = latp.tile([64, S], F32R)

        # ---------------- Phase A: sharded stem ----------------
        with (
            tc.tile_pool(name="shard", bufs=1) as shp,
            tc.tile_pool(name="stem_h", bufs=1) as hp,
            tc.tile_pool(name="stem_w", bufs=4) as wap,
            tc.tile_pool(name="sq", bufs=3) as sqp,
            tc.tile_pool(name="ssr", bufs=1) as ssrp,
            tc.tile_pool(name="stem_ps", bufs=1, space="PSUM") as psst,
        ):
            sh = [shp.tile([SH_W[t], S], F32R, name=f"sh{t}")
                  for t in range(5)]
            ssq_sb = ssrp.tile([1, S], F32R)
            sskv_sb = ssrp.tile([1, S], F32R)
            for i2 in range(NQSB):
                qs = slice(i2 * QSB, (i2 + 1) * QSB)
                pss = [psst.tile([SH_W[t], QSB], F32, name=f"ps_stem{t}",
                                 tag=f"s{t}") for t in range(5)]
                for e in range(NE):
                    hT_c = hp.tile([128, QSB], F32R, tag="ht", name="hT_c",
                                   bufs=6)
                    nc.sync.dma_start(out=hT_c, in_=hT_re[:, e, qs])
                    wa_t = wap.tile([128, SHW], F32R, tag="wa", name="wa_t")
                    nc.sync.dma_start(
                        out=wa_t, in_=wa_d[e * 128:(e + 1) * 128, :])
                    for t in range(5):
                        nc.tensor.matmul(
                            pss[t], wa_t[:, t * 128:t * 128 + SH_W[t]],
                            hT_c,
                            start=(e == 0), stop=(e == NE - 1))
                for t in range(5):
                    nc.vector.tensor_copy(sh[t][:, qs], pss[t])
                # local partial sum-of-squares (q: tiles 0-2, kv: tile 3)
                ssq = psst.tile([1, QSB], F32, tag="ssq", name="ssq")
                for t in range(3):
                    sq = sqp.tile([128, QSB], F32R, tag="sq", name="sq")
                    nc.scalar.activation(sq, sh[t][:, qs], Square)
                    nc.tensor.matmul(ssq, ones_r, sq,
                                     start=(t == 0), stop=(t == 2))
                sskv = psst.tile([1, QSB], F32, tag="sskv", name="sskv")
                sq3 = sqp.tile([128, QSB], F32R, tag="sq", name="sq3")
                nc.scalar.activation(sq3, sh[3][:, qs], Square)
                nc.tensor.matmul(sskv, ones_r, sq3, start=True, stop=True)
                nc.scalar.activation(ssq_sb[:, qs], ssq, Copy)
                nc.scalar.activation(sskv_sb[:, qs], sskv, Copy)
            # ship shard + partials to the AllGather buffer
            for t in range(5):
                nc.sync.dma_start(out=ag_in[t * 128:t * 128 + SH_W[t], :],
                                  in_=sh[t])
            nc.sync.dma_start(out=ag_in[SHW:SHW + 1, :], in_=ssq_sb)
            nc.sync.dma_start(out=ag_in[SHW + 1:SHW + 2, :], in_=sskv_sb)

        nc.gpsimd.collective_compute(
            "AllGather", mybir.AluOpType.bypass,
            replica_groups=[[0, 1, 2, 3], [4, 5, 6, 7]],
            ins=[ag_in], outs=[ag_out])

        # these loads need no latents — they fill the AllGather wait
        nc.sync.dma_start(out=coslo, in_=coslo_d)
        nc.sync.dma_start(out=coshi, in_=coshi_d)
        nc.sync.dma_start(out=sinlo, in_=sinlo_d)
        nc.sync.dma_start(out=sinhi, in_=sinhi_d)
        if n_pat:
            for p in range(n_pat):
                nc.sync.dma_start(out=maskp[:, p, :], in_=mp_d[p])
        for r in range(NRKV):
            nc.sync.dma_start(out=kvn_all[:, r, :],
                              in_=kvbn_d[r * 128:(r + 1) * 128, :])
            nc.sync.dma_start(out=kvv_all[:, r, :],
                              in_=kvbv_d[r * 128:(r + 1) * 128, :])

        # ---------------- Phase B: reassemble + rmsnorm rows ----------
        with (
            tc.tile_pool(name="rms", bufs=4) as rmsp,
            tc.tile_pool(name="rms_ps", bufs=2, space="PSUM") as psss,
        ):
            ssg = rmsp.tile([4, S], F32R, tag="ssg", name="ssg")
            sskvg = rmsp.tile([4, S], F32R, tag="sskvg", name="sskvg")
            nc.sync.dma_start(out=ssg, in_=ag_out[:, SHW, :])
            nc.sync.dma_start(out=sskvg, in_=ag_out[:, SHW + 1, :])
            for r in range(NRQ):
                g, lt = divmod(r, 3)
                nc.sync.dma_start(
                    out=cq_raw[r],
                    in_=ag_out[g, lt * 128:(lt + 1) * 128, :])
            for r in range(NRKV):
                nc.sync.dma_start(out=ckv_raw[r],
                                  in_=ag_out[r, 384:512, :])
            nc.sync.dma_start(out=krot_raw, in_=ag_out[0, 512:576, :])
            for side, src, dim in ((0, ssg, QLR), (1, sskvg, KVLR)):
                for i2 in range(NQSB):
                    qs = slice(i2 * QSB, (i2 + 1) * QSB)
                    ss = psss.tile([1, QSB], F32, tag="ss", name="ss")
                    nc.tensor.matmul(ss, ones_r[0:4, :], src[:, qs],
                                     start=True, stop=True)
                    srow = rmsp.tile([1, QSB], F32, tag="srow", name="srow")
                    nc.scalar.activation(srow, ss, Sqrt,
                                         scale=1.0 / dim, bias=eps_t)
                    vrow = rmsp.tile([1, QSB], F32, tag="vrow", name="vrow")
                    nc.scalar.activation(vrow, ss, Ident,
                                         scale=1.0 / dim, bias=eps_t)
                    r0 = rmsp.tile([1, QSB], F32, tag="r0", name="r0")
                    nc.vector.reciprocal(r0, srow)
                    # Newton: r1 = r0*(1.5 - 0.5*v*r0^2)
                    t1 = rmsp.tile([1, QSB], F32, tag="t1", name="t1")
                    nc.vector.tensor_mul(t1, r0, r0)
                    nc.vector.tensor_mul(t1, t1, vrow)
                    nc.vector.tensor_scalar(t1, t1, -0.5, 1.5, MUL, ADD)
                    nc.vector.tensor_mul(t1, t1, r0)
                    nc.sync.dma_start(out=rs_scr[side:side + 1, qs], in_=t1)

        # rope on k (headless): krot_raw rows lo 0:32, hi 32:64
        with tc.tile_pool(name="kr", bufs=1) as krp:
            ka1 = krp.tile([32, S], F32)
            ka2 = krp.tile([32, S], F32)
            nc.vector.tensor_mul(ka1, krot_raw[0:32], coslo[0:32])
            nc.vector.tensor_mul(ka2, krot_raw[32:64], sinlo[32:64])
            nc.vector.tensor_sub(krope[0:32], ka1, ka2)
            nc.vector.tensor_mul(ka1, krot_raw[32:64], coshi[32:64])
            nc.vector.tensor_mul(ka2, krot_raw[0:32], sinhi[0:32])
            nc.vector.tensor_add(krope[32:64], ka1, ka2)

        # ---------------- Phase C: per-head-pair q_b/kv_b/attn ----------
        with (
            tc.tile_pool(name="scales", bufs=1) as scp,
            tc.tile_pool(name="pairq", bufs=1) as pairq,
            tc.tile_pool(name="pairw", bufs=2) as pairw,
            tc.tile_pool(name="ropet", bufs=1) as ropet,
            tc.tile_pool(name="exps", bufs=4) as expp,
            tc.tile_pool(name="dn", bufs=2) as dnp,
            tc.tile_pool(name="ps_pj", bufs=1, space="PSUM") as pspj,
            tc.tile_pool(name="ps_sc", bufs=2, space="PSUM") as pssc,
            tc.tile_pool(name="ps_av", bufs=2, space="PSUM") as psav,
            tc.tile_pool(name="ps_dn", bufs=1, space="PSUM") as psdn,
        ):
            Rq = scp.tile([128, S], F32)
            nc.sync.dma_start(out=Rq, in_=rs_scr[0:1].to_broadcast([128, S]))
            Rkv = scp.tile([128, S], F32)
            nc.sync.dma_start(out=Rkv, in_=rs_scr[1:2].to_broadcast([128, S]))
            rkv_c = scp.tile([128, NKB, 1], F32)
            rs_colT = rs_scr[1:2].rearrange("o (kb p) -> kb p o", p=128)
            for kb in range(NKB):
                nc.sync.dma_start(out=rkv_c[:, kb, :], in_=rs_colT[kb])
            for hp2 in range(4):
                # --- q_b for the pair: m-tiles [nope0, nope1, lohi] ---
                qT_nope = [pairq.tile([128, S], F32R, tag=f"qTn{m}",
                                      name=f"qTn{m}") for m in range(2)]
                qlohi = pairq.tile([128, S], F32, tag="qlohi", name="qlohi")
                for i2 in range(NQSB):
                    qs = slice(i2 * QSB, (i2 + 1) * QSB)
                    pss = [pspj.tile([128, QSB], F32, tag=f"p{mi}",
                                     name=f"ps_qb{mi}") for mi in range(3)]
                    for r in range(NRQ):
                        qb_t = pairw.tile([128, 384], F32R, tag="qbw",
                                          name="qb_t")
                        nc.sync.dma_start(
                            out=qb_t,
                            in_=qbw_d[r * 128:(r + 1) * 128,
                                      hp2 * 384:(hp2 + 1) * 384])
                        for mi in range(3):
                            nc.tensor.matmul(
                                pss[mi], qb_t[:, mi * 128:(mi + 1) * 128],
                                cq_raw[r][:, qs],
                                start=(r == 0), stop=(r == NRQ - 1))
                    nc.vector.tensor_mul(qT_nope[0][:, qs], pss[0],
                                         Rq[:, qs])
                    nc.vector.tensor_mul(qT_nope[1][:, qs], pss[1],
                                         Rq[:, qs])
                    nc.vector.tensor_mul(qlohi[:, qs], pss[2], Rq[:, qs])
                # --- rope on q pair ---
                # qlohi rows: lo h0 0:32 | lo h1 32:64 | hi h0 64:96 | hi h1
                qrope = pairq.tile([64, 2, S], F32R, tag="qrope",
                                   name="qrope")
                a1 = ropet.tile([64, S], F32, tag="a1", name="a1")
                a2 = ropet.tile([64, S], F32, tag="a2", name="a2")
                nc.vector.tensor_mul(a1, qlohi[0:64], coslo[0:64])
                nc.vector.tensor_mul(a2, qlohi[64:128], sinlo[64:128])
                for hh in range(2):
                    nc.vector.tensor_sub(qrope[0:32, hh, :],
                                         a1[hh * 32:(hh + 1) * 32, :],
                                         a2[hh * 32:(hh + 1) * 32, :])
                nc.vector.tensor_mul(a1, qlohi[64:128], coshi[64:128])
                nc.vector.tensor_mul(a2, qlohi[0:64], sinhi[0:64])
                for hh in range(2):
                    nc.vector.tensor_add(qrope[32:64, hh, :],
                                         a1[hh * 32:(hh + 1) * 32, :],
                                         a2[hh * 32:(hh + 1) * 32, :])

                # --- kv_b for the pair (weights resident) ---
                k_passT = [pairq.tile([128, S], F32R, tag=f"kT{m}",
                                      name=f"kT{m}") for m in range(2)]
                for i2 in range(NQSB):
                    qs = slice(i2 * QSB, (i2 + 1) * QSB)
                    pk = [pspj.tile([128, QSB], F32, tag=f"p{mi}",
                                    name=f"ps_kv{mi}") for mi in range(2)]
                    for r in range(NRKV):
                        for mi in range(2):
                            nc.tensor.matmul(
                                pk[mi],
                                kvn_all[:, r, hp2 * 256 + mi * 128:hp2 * 256 + (mi + 1) * 128],
                                ckv_raw[r][:, qs],
                                start=(r == 0), stop=(r == NRKV - 1))
                    for mi in range(2):
                        nc.vector.tensor_mul(k_passT[mi][:, qs], pk[mi],
                                             Rkv[:, qs])
                v_p = pairq.tile([128, NKB, 2 * VH], F32R, tag="vp",
                                 name="v_p")
                for kb in range(NKB):
                    psv = pspj.tile([128, 2 * VH], F32, tag="p2", name="psv")
                    for r in range(NRKV):
                        nc.tensor.matmul(
                            psv, ckv_raw[r][:, kb * 128:(kb + 1) * 128],
                            kvv_all[:, r, hp2 * 256:(hp2 + 1) * 256],
                            start=(r == 0), stop=(r == NRKV - 1))
                    nc.vector.tensor_scalar_mul(v_p[:, kb, :], psv,
                                                rkv_c[:, kb, :])

                # --- attention for both heads of the pair ---
                for hh in range(2):
                    h = hp2 * 2 + hh
                    for i2 in range(NQSB):
                        qs = slice(i2 * QSB, (i2 + 1) * QSB)
                        js = [j for j in range(NKB)
                              if plan[i2][j] != 'skip']
                        ets = []
                        for j in js:
                            ps = pssc.tile([128, QSB], F32, tag="sc",
                                           name="ps_sc")
                            nc.tensor.matmul(
                                ps,
                                k_passT[hh][:, j * 128:(j + 1) * 128],
                                qT_nope[hh][:, qs], start=True, stop=False)
                            nc.tensor.matmul(
                                ps, krope[:, j * 128:(j + 1) * 128],
                                qrope[:, hh, qs], start=False, stop=True)
                            pat = plan[i2][j]
                            if pat is not None:
                                nc.vector.tensor_add(ps, ps,
                                                     maskp[:, pat, :])
                            et = expp.tile([128, QSB], F32R, tag="exp",
                                           name="et")
                            nc.scalar.activation(et, ps, Exp, scale=SCALING)
                            ets.append(et)
                        pa = psav.tile([128, QSB], F32, tag="av", name="pa")
                        pd = psdn.tile([1, QSB], F32, tag="dn", name="pd")
                        for n, (j, et) in enumerate(zip(js, ets)):
                            nc.tensor.matmul(
                                pa, v_p[:, j, hh * 128:hh * 128 + 128], et,
                                start=(n == 0), stop=(n == len(js) - 1))
                            nc.tensor.matmul(
                                pd, ones_r, et,
                                start=(n == 0), stop=(n == len(js) - 1))
                        rec = dnp.tile([1, QSB], F32, tag="rec", name="rec")
                        nc.vector.reciprocal(rec, pd)
                        sl = h * NQSB + i2
                        nc.sync.dma_start(out=den_scr[sl:sl + 1, :], in_=rec)
                        bc = dnp.tile([128, QSB], F32, tag="bc", name="bc")
                        nc.sync.dma_start(
                            out=bc,
                            in_=den_scr[sl:sl + 1, :].to_broadcast(
                                [128, QSB]))
                        ao = dnp.tile([128, QSB], F32R, tag="ao", name="ao")
                        nc.vector.tensor_mul(ao, pa, bc)
                        nc.sync.dma_start(out=attn_scr[h][:, qs], in_=ao)

        _latp_cm.__exit__(None, None, None)

        # ---------------- Phase D: o_proj ----------------
        with (
            tc.tile_pool(name="ow", bufs=1) as owp,
            tc.tile_pool(name="ats", bufs=3) as atsp,
            tc.tile_pool(name="ob", bufs=4) as obp,
            tc.tile_pool(name="ps_o", bufs=4, space="PSUM") as pso,
        ):
            EH = HID // 2  # 2048 per half
            for eh in range(2):
                ow_sb = owp.tile([128, HG, EH], F32R, tag="ow", name="ow_sb")
                for c in range(HG):
                    nc.sync.dma_start(
                        out=ow_sb[:, c, :],
                        in_=ow_d[c * 128:(c + 1) * 128,
                                 eh * EH:(eh + 1) * EH])
                for i in range(NKB):
                    at_i = atsp.tile([128, HG, 128], F32R, tag="at",
                                     name="at_i")
                    for c in range(HG):
                        nc.sync.dma_start(
                            out=at_i[:, c, :],
                            in_=attn_scr[c][:, i * 128:(i + 1) * 128])
                    for es in range(EH // QSB):
                        po = pso.tile([128, QSB], F32, tag="po", name="po")
                        for c in range(HG):
                            nc.tensor.matmul(
                                po, at_i[:, c, :],
                                ow_sb[:, c, es * QSB:(es + 1) * QSB],
                                start=(c == 0), stop=(c == HG - 1))
                        ob = obp.tile([128, QSB], F32, tag="ob", name="ob")
                        nc.scalar.copy(ob, po)
                        nc.sync.dma_start(
                            out=out_d[i * 128:(i + 1) * 128,
                                      eh * EH + es * QSB:
                                      eh * EH + (es + 1) * QSB],
                            in_=ob)
    nc.compile()
    return nc


def _prep_inputs(hidden_states, cos, sin, attention_mask, q_a_w, q_a_ln_w,
                 q_b_w, kv_a_w, kv_a_ln_w, kv_b_w, o_w):
    """Build the 8 per-core input maps + the mask plan."""
    f = np.float32
    plan, patterns = _mask_plan(np.asarray(attention_mask, f))

    q_a_w = np.asarray(q_a_w, f)
    kv_a_w = np.asarray(kv_a_w, f)
    qbw_full = np.asarray(q_a_ln_w, f)[:, None] * np.asarray(q_b_w, f)
    kvb_full = np.asarray(kv_a_ln_w, f)[:, None] * np.asarray(kv_b_w, f)
    kvb_full = kvb_full.reshape(KVLR, H, NOPE + VH)
    o_w = np.asarray(o_w, f)

    per_batch = []
    for b in range(B):
        hT = np.ascontiguousarray(np.asarray(hidden_states, f)[b].T)
        cosT = np.asarray(cos, f)[b].T  # [ROPE, S]
        sinT = np.asarray(sin, f)[b].T
        per_batch.append(dict(
            hT=hT,
            coslo=np.ascontiguousarray(np.tile(cosT[:32], (4, 1))),
            coshi=np.ascontiguousarray(np.tile(cosT[32:], (4, 1))),
            sinlo=np.ascontiguousarray(np.tile(sinT[:32], (4, 1))),
            sinhi=np.ascontiguousarray(np.tile(sinT[32:], (4, 1))),
        ))

    in_maps = []
    for c in range(NCORES):
        b, g = divmod(c, 4)
        hs = g * HG  # first head of this core
        # stem shard weights: 3 q-tiles + 1 kv-tile + (krot | zeros)
        extra = (kv_a_w[:, KVLR:KVLR + ROPE] if g == 0
                 else np.zeros((HID, 64), f))
        wa = np.ascontiguousarray(np.concatenate(
            [q_a_w[:, 384 * g:384 * (g + 1)],
             kv_a_w[:, 128 * g:128 * (g + 1)], extra], axis=1))
        qb = qbw_full[:, hs * QKH:(hs + HG) * QKH].reshape(QLR, HG, QKH)
        cols = []
        for p in range(4):  # head pairs
            h0, h1 = 2 * p, 2 * p + 1
            cols += [qb[:, h0, :NOPE], qb[:, h1, :NOPE],
                     qb[:, h0, NOPE:NOPE + 32], qb[:, h1, NOPE:NOPE + 32],
                     qb[:, h0, NOPE + 32:], qb[:, h1, NOPE + 32:]]
        qbw = np.ascontiguousarray(np.concatenate(cols, axis=1))
        kvb = kvb_full[:, hs:hs + HG]
        kvbn = np.ascontiguousarray(kvb[:, :, :NOPE].reshape(KVLR, HG * NOPE))
        kvbv = np.ascontiguousarray(kvb[:, :, NOPE:].reshape(KVLR, HG * VH))
        ow = np.ascontiguousarray(o_w[hs * VH:(hs + HG) * VH])
        m = dict(per_batch[b])
        m.update(wa=wa, qbw=qbw, kvbn=kvbn, kvbv=kvbv, ow=ow,
                 ones=np.ones((128, 1), f))
        if patterns:
            m["maskpat"] = np.ascontiguousarray(np.stack(patterns))
        in_maps.append(m)
    return in_maps, plan, patterns


def kernel(**inputs):
    from concourse import bass_utils

    in_maps, plan, patterns = _prep_inputs(**inputs)
    key = str(plan)
    if key not in _CACHE:
        _CACHE[key] = _build(plan, len(patterns))
    nc = _CACHE[key]
    res = bass_utils.run_bass_kernel_spmd(nc, in_maps,
                                          core_ids=list(range(NCORES)))
    out = np.zeros((B, S, HID), np.float32)
    for c in range(NCORES):
        out[c // 4] += res.results[c]["outp"]
    return out
